# revision 10
# baseline (speedup 1.0000x reference)
"""Trainium2 Bass kernel for a rate-1/2, constraint-length-3 feedforward
convolutional encoder (generator polynomials "101" and "111", MSB-first).

The trellis scan in the reference collapses to elementwise XORs of shifted
input bits (zero initial state):

    out0[t] = u[t] ^ u[t-2]            (poly "101")
    out1[t] = u[t] ^ u[t-1] ^ u[t-2]   (poly "111")

with the codeword interleaved time-major: y[:, 2t] = out0[t], y[:, 2t+1] = out1[t].

XOR on {0,1} f32 values is computed bitwise on uint32 views (1.0f =
0x3F800000, 0.0f = 0x0), one DVE op per output stream.

DMA layout: the kernel is bound by the ~430 GB/s/core SDMA-engine
aggregate (16 engines x ~27 GiB/s); total traffic is fixed at 24 MiB
per core (8 in + 16 out). Inputs are issued upfront, alternating across
both HWDGE rings (SP + ACT), which together sustain ~427 GB/s on reads
(a single ring caps near ~360). Outputs stream on the SWDGE (gpsimd)
queue, which sustains ~433 GB/s solo; it naturally starts once the read
phase is draining, which measured best — forcing writes earlier makes
the 3-queue round-robin drop well below the engine cap. All 8 input
slots and 8 output tiles are SBUF-resident, so no DMA ever waits on
buffer recycling.

Sharding: pure data parallel over the batch dim across 8 NeuronCores.
"""

import numpy as np

N_CORES = 8
B, K = 8192, 2048
N_OUT = 2
SHARD_B = B // N_CORES  # 1024 codewords per core
P = 128                 # SBUF partitions

_compiled = {}


def _build_nc():
    import concourse.bass as bass  # noqa: F401
    import concourse.tile as tile
    from concourse import bacc, mybir

    nc = bacc.Bacc(
        "TRN2",
        target_bir_lowering=False,
        debug=False,
        enable_asserts=False,
    )
    x = nc.dram_tensor("x", [SHARD_B, K], mybir.dt.float32, kind="ExternalInput").ap()
    y = nc.dram_tensor(
        "y", [SHARD_B, N_OUT * K], mybir.dt.float32, kind="ExternalOutput"
    ).ap()

    n_groups = SHARD_B // P  # 8 row-groups of 128

    with tile.TileContext(nc) as tc:
        with (
            tc.tile_pool(name="xin", bufs=1) as in_pool,
            tc.tile_pool(name="out", bufs=1) as out_pool,
        ):
            # Persistent input slots with 2 leading zero columns so the
            # shifted views u[t-1], u[t-2] fall out of plain column offsets.
            in_slots = [
                in_pool.tile(
                    [P, K + 2], mybir.dt.float32, tag=f"xin{j}", name=f"xin{j}"
                )
                for j in range(n_groups)
            ]
            out_slots = [
                out_pool.tile(
                    [P, N_OUT * K], mybir.dt.float32, tag=f"out{j}", name=f"out{j}"
                )
                for j in range(n_groups)
            ]
            for j in range(n_groups):
                nc.vector.memset(in_slots[j][:, 0:2], 0.0)

            # All input DMAs upfront, alternating between the two HWDGE
            # rings (SP and ACT) so read descriptors stream from two
            # independent queues. The first DMA on each ring covers only
            # 32 partitions: HWDGE descriptor generation is per-descriptor
            # (one per partition run), and the shared RTL serializes the
            # rings' first sets — a small first set gets both rings
            # streaming ~1.5us sooner.
            PSPLIT = 32
            nc.sync.dma_start(in_slots[0][0:PSPLIT, 2 : 2 + K], x[0:PSPLIT, :])
            nc.scalar.dma_start(
                in_slots[1][0:PSPLIT, 2 : 2 + K], x[P : P + PSPLIT, :]
            )
            nc.sync.dma_start(in_slots[0][PSPLIT:P, 2 : 2 + K], x[PSPLIT:P, :])
            nc.scalar.dma_start(
                in_slots[1][PSPLIT:P, 2 : 2 + K], x[P + PSPLIT : 2 * P, :]
            )
            for g in range(2, n_groups):
                rows = slice(g * P, (g + 1) * P)
                eng = nc.sync if g % 2 == 0 else nc.scalar
                eng.dma_start(in_slots[g][:, 2 : 2 + K], x[rows, :])

            for g in range(n_groups):
                xin = in_slots[g]
                rows = slice(g * P, (g + 1) * P)
                a = xin[:, 2 : 2 + K].bitcast(mybir.dt.uint32)  # u[t]
                b = xin[:, 1 : 1 + K].bitcast(mybir.dt.uint32)  # u[t-1]
                c = xin[:, 0:K].bitcast(mybir.dt.uint32)        # u[t-2]

                out = out_slots[g]
                even = out[:, 0 : N_OUT * K : 2].bitcast(mybir.dt.uint32)
                odd = out[:, 1 : N_OUT * K : 2].bitcast(mybir.dt.uint32)

                # out0 = a ^ c ; out1 = out0 ^ b  (bitwise on f32 payloads)
                nc.vector.tensor_tensor(even, a, c, mybir.AluOpType.bitwise_xor)
                nc.vector.tensor_tensor(odd, even, b, mybir.AluOpType.bitwise_xor)

                # Early outputs (g0-g3) stream on SWDGE: its first Q7
                # emission is slow under DVE contention, so it must get the
                # earliest-ready work to be flowing by the time reads
                # drain. Late outputs (g4-g7) ride the HWDGE rings, whose
                # FIFOs roll straight from reads into writes and fill the
                # read->write transition dip.
                if g < 4:
                    nc.gpsimd.dma_start(y[rows, :], out[:])
                else:
                    oeng = nc.sync if g % 2 == 0 else nc.scalar
                    oeng.dma_start(y[rows, :], out[:])

    nc.compile()
    return nc


def _get_nc():
    if "nc" not in _compiled:
        _compiled["nc"] = _build_nc()
    return _compiled["nc"]


def kernel(**inputs) -> np.ndarray:
    from concourse.bass_utils import run_bass_kernel_spmd

    x_full = np.ascontiguousarray(np.asarray(inputs["inputs"], dtype=np.float32))
    assert x_full.shape == (B, K), x_full.shape

    nc = _get_nc()
    in_maps = [
        {"x": x_full[i * SHARD_B : (i + 1) * SHARD_B]} for i in range(N_CORES)
    ]
    res = run_bass_kernel_spmd(nc, in_maps, core_ids=list(range(N_CORES)))
    out = np.concatenate([r["y"] for r in res.results], axis=0)
    return np.ascontiguousarray(out, dtype=np.float32)
